# revision 1
# baseline (speedup 1.0000x reference)
"""MoE LoRA linear layer kernel for Trainium2, data-parallel over 8 NeuronCores.

Math (per token n):
    down = h @ down_w.T                      [N, 64]
    mask[n, r] = val[n, k] if idx[n, k] == r else 0   (indices distinct per row)
    out = (down * mask) @ up_w.T             [N, 4096]

Sharding: tokens split 8 ways (2048/core); LoRA weights replicated.

Per-core pipeline (token tile TT=256 = 2 chunks of 128):
  1. load h in natural layout [128, 4096] per chunk (16KB DMA descriptors;
     a strided transpose-load would be 512B/descriptor and bottleneck the
     sync engine on descriptor generation)
  2. PE-transpose h blocks, 4 per PSUM bank, one fat [128, 512] copy each
     (copies alternate DVE/ACT)
  3. 32 f32r matmuls accumulate downT = dwT.T @ hT into PSUM [64, 256]
  4. top-k scatter mask: 8x tensor_scalar one-hot*val on DVE, transposed
     into one PSUM bank with matmul accumulation (no DVE adds), multiply
     with downT -> resT
  5. up-proj per chunk: 8x f32r matmul [K=64, M=128, N=512] -> psum,
     assemble out_sb [128, 4096], single fat store per chunk

f32r (4-byte storage, reduced-precision PE multiply) runs matmuls at 1
cycle/row for free dims >= 256 vs 4 cycles/row for plain fp32.

All small constants (dwT, identity, iota, idx, val) are host-packed into one
[128, CB] blob = single DMA.
"""

import sys

for p in ("/opt/trn_rl_repo", "/opt/pypackages"):
    if p not in sys.path:
        sys.path.insert(0, p)

import numpy as np

N, D_IN, D_OUT, RANK, TOPK = 16384, 4096, 4096, 64, 8
NCORES = 8
NT = N // NCORES          # tokens per core = 2048
P = 128                   # partitions
TT = 256                  # token tile (down-matmul free dim)
NKC = D_IN // P           # 32 contraction chunks for down proj
NJ = TT // P              # 2 x 128-token chunks per tile
NTILES = NT // TT         # 8 token tiles per core
NCHUNK = NT // P          # 16 x 128-token chunks per core
OT = 512                  # output col tile
NOT = D_OUT // OT         # 8 output col tiles

# const blob column layout (f32, [128, CB])
C_DWT = 0                 # [128, 32*64]   dwT chunk ki at C_DWT + ki*64
C_ID = C_DWT + NKC * RANK           # [128, 128] identity
C_IOTA = C_ID + P                   # [128, 64]  iota over rank
C_IDX = C_IOTA + RANK               # [128, 16*8] idx (chunk-major)
C_VAL = C_IDX + NCHUNK * TOPK       # [128, 16*8] val
CB = C_VAL + NCHUNK * TOPK

_CACHE = {}


def _build_program():
    import concourse.bacc as bacc
    import concourse.mybir as mybir
    from concourse import tile

    f32 = mybir.dt.float32
    f32r = mybir.dt.float32r
    # Bacc (not plain Bass): its finalize() runs move_matmul_waits_to_-
    # ldweights + generate_event_semaphores, which split semaphore waits to
    # satisfy the TRN2 one-wait-per-instruction constraint.
    nc = bacc.Bacc()

    h = nc.declare_dram_parameter("h", [NT, D_IN], f32, isOutput=False)
    cblob = nc.declare_dram_parameter("cblob", [P, CB], f32, isOutput=False)
    upw = nc.declare_dram_parameter("upw", [RANK, D_OUT], f32, isOutput=False)
    out = nc.declare_dram_parameter("out", [NT, D_OUT], f32, isOutput=True)

    eq = mybir.AluOpType.is_equal
    mult = mybir.AluOpType.mult

    with tile.TileContext(nc) as tc:
        with (
            tc.tile_pool(name="const", bufs=1) as const,
            tc.tile_pool(name="hnat", bufs=3) as hnat_pool,
            tc.tile_pool(name="hT", bufs=2) as hT_pool,
            tc.tile_pool(name="mask", bufs=4) as mask_pool,
            tc.tile_pool(name="resT", bufs=2) as resT_pool,
            tc.tile_pool(name="outsb", bufs=2) as out_pool,
            tc.tile_pool(name="psum_h", bufs=2, space="PSUM") as psum_h_pool,
            tc.tile_pool(name="psum_dn", bufs=2, space="PSUM") as psum_dn_pool,
            tc.tile_pool(name="psum_up", bufs=2, space="PSUM") as psum_up_pool,
            tc.tile_pool(name="psum_trm", bufs=2, space="PSUM") as psum_trm_pool,
        ):
            cb = const.tile([P, CB], f32)
            upT = const.tile([RANK, D_OUT], f32)

            nc.sync.dma_start(out=cb[:], in_=cblob[:, :])
            nc.sync.dma_start(out=upT[:], in_=upw[:, :])

            # f32r operands must come from a rounding producer; DMA can't
            # round, so copy the weights into f32r tiles once.
            dwT_r = const.tile([P, NKC * RANK], f32r)
            upT_r = const.tile([RANK, D_OUT], f32r)
            nc.vector.tensor_copy(out=dwT_r[:], in_=cb[:, C_DWT:C_DWT + NKC * RANK])
            nc.scalar.copy(out=upT_r[:], in_=upT[:])

            dwT = cb[:, C_DWT:C_DWT + NKC * RANK]
            ident = cb[:, C_ID:C_ID + P]
            iota_sb = cb[:, C_IOTA:C_IOTA + RANK]
            idx_sb = cb[:, C_IDX:C_IDX + NCHUNK * TOPK]
            val_sb = cb[:, C_VAL:C_VAL + NCHUNK * TOPK]

            copy_engines = [nc.vector.tensor_copy, nc.scalar.copy]
            cp_i = 0

            for tt in range(NTILES):
                # 1. natural-layout loads, one per 128-token chunk
                h_nats = []
                for j in range(NJ):
                    h_nat = hnat_pool.tile([P, D_IN], f32)
                    row = tt * TT + j * P
                    nc.sync.dma_start(out=h_nat[:], in_=h[row:row + P, :])
                    h_nats.append(h_nat)

                # 2. PE-transpose h blocks into hT; 4 transposes (2 ki x 2 j)
                #    share one PSUM bank -> one fat [128, 512] copy
                hT = hT_pool.tile([P, NKC * TT], f32r)
                for kb in range(NKC // 2):
                    psum_h = psum_h_pool.tile([P, 2 * TT], f32)
                    for ki2 in range(2):
                        ki = kb * 2 + ki2
                        for j in range(NJ):
                            nc.tensor.transpose(
                                psum_h[:, ki2 * TT + j * P:ki2 * TT + (j + 1) * P],
                                h_nats[j][:, ki * P:(ki + 1) * P],
                                ident[:],
                            )
                    cp = copy_engines[cp_i % 2]
                    cp_i += 1
                    cp(
                        out=hT[:, kb * 2 * TT:(kb + 1) * 2 * TT],
                        in_=psum_h[:],
                    )

                # 3. down projection, accumulated over NKC chunks (f32r)
                psum_dn = psum_dn_pool.tile([RANK, TT], f32)
                for ki in range(NKC):
                    nc.tensor.matmul(
                        psum_dn[:],
                        lhsT=dwT_r[:, ki * RANK:(ki + 1) * RANK],
                        rhs=hT[:, ki * TT:(ki + 1) * TT],
                        start=(ki == 0),
                        stop=(ki == NKC - 1),
                    )

                # psum_dn -> SBUF so the mask multiply has one PSUM operand
                down_sb = resT_pool.tile([RANK, TT], f32, tag="down_sb")
                nc.scalar.copy(out=down_sb[:], in_=psum_dn[:])

                resT = resT_pool.tile([RANK, TT], f32r)
                for j in range(NJ):
                    jj = tt * NJ + j
                    # 4. top-k scatter mask: one-hot*val per k on DVE, summed
                    #    in PSUM via accumulating transpose matmuls
                    psum_tr = psum_trm_pool.tile([RANK, P], f32)
                    for k in range(TOPK):
                        col = jj * TOPK + k
                        oh = mask_pool.tile([P, RANK], f32)
                        nc.vector.tensor_scalar(
                            out=oh[:],
                            in0=iota_sb[:],
                            scalar1=idx_sb[:, col:col + 1],
                            scalar2=val_sb[:, col:col + 1],
                            op0=eq,
                            op1=mult,
                        )
                        nc.tensor.matmul(
                            psum_tr[:],
                            lhsT=oh[:],
                            rhs=ident[:],
                            is_transpose=True,
                            start=(k == 0),
                            stop=(k == TOPK - 1),
                        )
                    nc.vector.tensor_mul(
                        resT[:, j * P:(j + 1) * P],
                        down_sb[:, j * P:(j + 1) * P],
                        psum_tr[:],
                    )

                    # 5. up projection (f32r) + fat store
                    out_sb = out_pool.tile([P, D_OUT], f32)
                    for o in range(NOT):
                        psum_up = psum_up_pool.tile([P, OT], f32)
                        nc.tensor.matmul(
                            psum_up[:],
                            lhsT=resT[:, j * P:(j + 1) * P],
                            rhs=upT_r[:, o * OT:(o + 1) * OT],
                            start=True,
                            stop=True,
                        )
                        cp = copy_engines[cp_i % 2]
                        cp_i += 1
                        cp(
                            out=out_sb[:, o * OT:(o + 1) * OT],
                            in_=psum_up[:],
                        )
                    nc.sync.dma_start(
                        out=out[jj * P:(jj + 1) * P, :],
                        in_=out_sb[:],
                    )

    # Run the Bacc pipeline (register alloc + wait splitting for the TRN2
    # one-wait-per-instruction constraint) before the module is serialized.
    nc.finalize()
    return nc


def _get_program():
    if "nc" not in _CACHE:
        _CACHE["nc"] = _build_program()
    return _CACHE["nc"]


def prepare_in_maps(hidden_states, down_w, up_w, top_k_values, top_k_indices):
    h = np.ascontiguousarray(hidden_states, dtype=np.float32)
    dw = np.ascontiguousarray(down_w, dtype=np.float32)
    uw = np.ascontiguousarray(up_w, dtype=np.float32)
    vals = np.ascontiguousarray(top_k_values, dtype=np.float32)
    idxf = top_k_indices.astype(np.float32)

    upT = np.ascontiguousarray(uw.T)  # [64, 4096]

    # dwT[i, kc*64 + r] = dw[r, kc*128 + i]
    dwT = dw.reshape(RANK, NKC, P).transpose(2, 1, 0).reshape(P, NKC * RANK)
    ident = np.eye(P, dtype=np.float32)
    iota = np.broadcast_to(np.arange(RANK, dtype=np.float32), (P, RANK))

    in_maps = []
    for c in range(NCORES):
        s = slice(c * NT, (c + 1) * NT)
        # idx/val packed [p, chunk*8 + k] for this core's 16 chunks
        idx_p = idxf[s].reshape(NCHUNK, P, TOPK).transpose(1, 0, 2).reshape(P, -1)
        val_p = vals[s].reshape(NCHUNK, P, TOPK).transpose(1, 0, 2).reshape(P, -1)
        cb = np.concatenate([dwT, ident, iota, idx_p, val_p], axis=1)
        assert cb.shape == (P, CB)
        in_maps.append(
            {
                "h": h[s],
                "cblob": np.ascontiguousarray(cb),
                "upw": upT,
            }
        )
    return in_maps


def kernel(hidden_states, down_w, up_w, top_k_values, top_k_indices, **_kw):
    from concourse.bass_utils import run_bass_kernel_spmd

    nc = _get_program()
    in_maps = prepare_in_maps(
        hidden_states, down_w, up_w, top_k_values, top_k_indices
    )
    res = run_bass_kernel_spmd(nc, in_maps, core_ids=list(range(NCORES)))
    return np.concatenate([r["out"] for r in res.results], axis=0)



# revision 3
# speedup vs baseline: 1.7150x; 1.7150x over previous
"""MoE LoRA linear layer kernel for Trainium2, data-parallel over 8 NeuronCores.

Math (per token n):
    down = h @ down_w.T                      [N, 64]
    mask[n, r] = val[n, k] if idx[n, k] == r else 0   (indices distinct per row)
    out = (down * mask) @ up_w.T             [N, 4096]

Sharding: tokens split 8 ways (2048/core); LoRA weights replicated.

Strategy (v2): the device does ONLY the two matmuls + one elementwise
multiply; everything layout-shaped moves to the host packer, and all
device traffic is bf16 (accumulation stays f32 in PSUM):

  * h is pre-transposed and pre-chunked on the host into
    ht[(half*32+ki)*128 + i, n] so the down contraction streams natural
    contiguous 2 KiB DMA lines -- no on-device PE transposes at all.
  * the top-k scatter mask is materialized densely on the host as
    maskT [64, 2048] f32 (tiny), so masking is a single DVE multiply
    fused with the psum->sbuf eviction of the down projection.
  * the up projection computes outT = up_w @ resT with the up weights
    STATIONARY across all token tiles (32 ldweights instead of 128),
    writing the output transposed; the host un-transposes when it
    gathers the 8 shards.
  * tokens are processed in 2 halves of 1024 so the out-DMA stream of
    half 0 overlaps the in-DMA stream of half 1 -- the DMA engines (the
    roofline: ~33.5 MiB/core at 360 GB/s ~= 95 us) never idle.

Engine budget per core (full pstate): DMA ~95 us (bound), PE ~55 us,
ACT/DVE ~45 us each, SP sequencer ~40 us.
"""

import sys

for p in ("/opt/trn_rl_repo", "/opt/pypackages"):
    if p not in sys.path:
        sys.path.insert(0, p)

import ml_dtypes
import numpy as np

BF16 = ml_dtypes.bfloat16

N, D_IN, D_OUT, RANK, TOPK = 16384, 4096, 4096, 64, 8
NCORES = 8
NT = N // NCORES          # tokens per core = 2048
P = 128                   # partitions
NKC = D_IN // P           # 32 contraction chunks for the down proj
HALF = NT // 2            # 1024 tokens per half
QW = 512                  # matmul free width (one PSUM bank of f32)
NQ = HALF // QW           # 2 free-dim tiles per half
NOC = D_OUT // P          # 32 output-row chunks for the up proj

_CACHE = {}


def _build_program():
    import concourse.bacc as bacc
    import concourse.mybir as mybir
    from concourse import tile

    f32 = mybir.dt.float32
    bf16 = mybir.dt.bfloat16
    nc = bacc.Bacc()

    ht = nc.declare_dram_parameter("ht", [2 * NKC * P, HALF], bf16, isOutput=False)
    dwt = nc.declare_dram_parameter("dwt", [P, NKC * RANK], bf16, isOutput=False)
    upw = nc.declare_dram_parameter("upw", [RANK, D_OUT], bf16, isOutput=False)
    maskt = nc.declare_dram_parameter("maskt", [RANK, NT], f32, isOutput=False)
    outt = nc.declare_dram_parameter("outt", [D_OUT, NT], bf16, isOutput=True)

    with tile.TileContext(nc) as tc:
        with (
            tc.tile_pool(name="const", bufs=1) as const,
            tc.tile_pool(name="hch", bufs=6) as hch_pool,
            tc.tile_pool(name="res", bufs=2) as res_pool,
            tc.tile_pool(name="outsb", bufs=3) as out_pool,
            tc.tile_pool(name="psum_dn", bufs=1, space="PSUM") as psum_dn_pool,
            tc.tile_pool(name="psum_up", bufs=3, space="PSUM") as psum_up_pool,
        ):
            dwt_sb = const.tile([P, NKC * RANK], bf16, name="dwt_sb")
            upw_sb = const.tile([RANK, D_OUT], bf16, name="upw_sb")
            maskt_sb = const.tile([RANK, NT], f32, name="maskt_sb")
            nc.sync.dma_start(out=dwt_sb[:], in_=dwt[:, :])
            nc.sync.dma_start(out=upw_sb[:], in_=upw[:, :])
            nc.sync.dma_start(out=maskt_sb[:], in_=maskt[:, :])

            # down-proj accumulators: [64, 512] f32 = one PSUM bank each;
            # 2 per half, both halves live at once -> 4 banks
            dn = [
                [
                    psum_dn_pool.tile([RANK, QW], f32, name=f"dn_{h}_{q}")
                    for q in range(NQ)
                ]
                for h in range(2)
            ]

            def emit_a_iter(h, ki):
                # one contraction chunk: load hT[ki] for this half, then
                # accumulate downT += dwT[ki].T @ hT[ki] into both q tiles
                hc = hch_pool.tile([P, HALF], bf16, name="hc")
                r0 = (h * NKC + ki) * P
                nc.sync.dma_start(out=hc[:], in_=ht[r0:r0 + P, :])
                for q in range(NQ):
                    nc.tensor.matmul(
                        dn[h][q][:],
                        lhsT=dwt_sb[:, ki * RANK:(ki + 1) * RANK],
                        rhs=hc[:, q * QW:(q + 1) * QW],
                        start=(ki == 0),
                        stop=(ki == NKC - 1),
                    )

            def emit_mask(h):
                # evict downT psum -> sbuf bf16, fused with the top-k mask
                resT = res_pool.tile([RANK, HALF], bf16, name="resT")
                for q in range(NQ):
                    col = h * HALF + q * QW
                    nc.vector.tensor_mul(
                        resT[:, q * QW:(q + 1) * QW],
                        maskt_sb[:, col:col + QW],
                        dn[h][q][:],
                    )
                return resT

            def emit_b_iter(h, oc, resT, store_eng):
                # one output-row chunk: outT[oc] = upT[oc].T @ resT,
                # psum -> sbuf bf16 (ACT + DVE), single 256 KiB store
                osb = out_pool.tile([P, HALF], bf16, name="osb")
                for q in range(NQ):
                    pu = psum_up_pool.tile([P, QW], f32, name="pu")
                    nc.tensor.matmul(
                        pu[:],
                        lhsT=upw_sb[:, oc * P:(oc + 1) * P],
                        rhs=resT[:, q * QW:(q + 1) * QW],
                        start=True,
                        stop=True,
                    )
                    if q % 2 == 0:
                        nc.scalar.copy(out=osb[:, q * QW:(q + 1) * QW], in_=pu[:])
                    else:
                        nc.vector.tensor_copy(
                            out=osb[:, q * QW:(q + 1) * QW], in_=pu[:]
                        )
                store_eng.dma_start(
                    out=outt[oc * P:(oc + 1) * P, h * HALF:(h + 1) * HALF],
                    in_=osb[:],
                )

            # half 0 down
            for ki in range(NKC):
                emit_a_iter(0, ki)
            res0 = emit_mask(0)
            # half 1 down interleaved with half 0 up: PE alternates the two
            # streams, DMA engines see loads+stores back to back
            for step in range(NKC):
                emit_a_iter(1, step)
                emit_b_iter(0, step, res0, nc.scalar)
            res1 = emit_mask(1)
            for oc in range(NOC):
                emit_b_iter(1, oc, res1, nc.sync)

    nc.finalize()
    return nc


def _get_program():
    if "nc" not in _CACHE:
        _CACHE["nc"] = _build_program()
    return _CACHE["nc"]


def prepare_in_maps(hidden_states, down_w, up_w, top_k_values, top_k_indices):
    h = np.ascontiguousarray(hidden_states, dtype=np.float32).astype(BF16)
    dw = np.ascontiguousarray(down_w, dtype=np.float32).astype(BF16)
    uw = np.ascontiguousarray(up_w, dtype=np.float32).astype(BF16)
    vals = np.ascontiguousarray(top_k_values, dtype=np.float32)
    idx = np.asarray(top_k_indices).astype(np.int64)

    # dwt[i, ki*64 + r] = dw[r, ki*128 + i]
    dwt = np.ascontiguousarray(
        dw.reshape(RANK, NKC, P).transpose(2, 1, 0).reshape(P, NKC * RANK)
    )
    upT = np.ascontiguousarray(uw.T)  # [64, 4096]

    rows = np.arange(NT)[:, None]
    in_maps = []
    for c in range(NCORES):
        s = slice(c * NT, (c + 1) * NT)
        # ht[(half*32+ki)*128 + i, n] = h[s][half*1024 + n, ki*128 + i]
        ht = np.ascontiguousarray(
            h[s].reshape(2, HALF, NKC, P).transpose(0, 2, 3, 1).reshape(-1, HALF)
        )
        m = np.zeros((NT, RANK), dtype=np.float32)
        m[rows, idx[s]] = vals[s]
        in_maps.append(
            {
                "ht": ht,
                "dwt": dwt,
                "upw": upT,
                "maskt": np.ascontiguousarray(m.T),
            }
        )
    return in_maps


def gather_output(results):
    # each core returns outT [4096, 2048] bf16; un-transpose + upcast
    return np.concatenate(
        [np.asarray(r["outt"]).T.astype(np.float32) for r in results], axis=0
    )


def kernel(hidden_states, down_w, up_w, top_k_values, top_k_indices, **_kw):
    from concourse.bass_utils import run_bass_kernel_spmd

    nc = _get_program()
    in_maps = prepare_in_maps(
        hidden_states, down_w, up_w, top_k_values, top_k_indices
    )
    res = run_bass_kernel_spmd(nc, in_maps, core_ids=list(range(NCORES)))
    return gather_output(res.results)


# revision 4
# speedup vs baseline: 1.9875x; 1.1589x over previous
"""MoE LoRA linear layer kernel for Trainium2, data-parallel over 8 NeuronCores.

Math (per token n):
    down = h @ down_w.T                      [N, 64]
    mask[n, r] = val[n, k] if idx[n, k] == r else 0   (indices distinct per row)
    out = (down * mask) @ up_w.T             [N, 4096]

Sharding: tokens split 8 ways (2048/core); LoRA weights replicated.

Strategy (v3): device does two matmul passes + one fused DVE multiply;
all layout work happens in the host packer, all traffic is bf16
(accumulation in f32 PSUM). DMA roofline ~34 MiB/core @ 360 GB/s ~= 98 us.

  * h is pre-transposed on the host (ht[ki*128+i, n]) so the down
    contraction streams contiguous 4 KiB DMA lines; no PE transposes.
  * down-proj: even ki chunks write PSUM partitions 0-63, odd ki
    chunks partitions 64-127 (128x64 column-tiled array mode, two
    concurrent tile streams). The even/odd partial sums are never
    added explicitly:
  * up-proj contracts K=128 against host-duplicated up weights
    (upw2 = [upT; upT]), so sum over both partial banks happens inside
    the matmul -- full 128x128 array, FWL weight loads.
  * the top-k scatter mask is a dense host-built maskT (bf16,
    replicated to 128 partitions); masking fuses with the PSUM->SBUF
    eviction of the down projection on the DVE.
  * up-proj emits outT = up_w @ resT (output transposed, stationary
    weights); the host un-transposes while gathering the 8 shards.
"""

import sys

for p in ("/opt/trn_rl_repo", "/opt/pypackages"):
    if p not in sys.path:
        sys.path.insert(0, p)

import ml_dtypes
import numpy as np

BF16 = ml_dtypes.bfloat16

N, D_IN, D_OUT, RANK, TOPK = 16384, 4096, 4096, 64, 8
NCORES = 8
NT = N // NCORES          # tokens per core = 2048
P = 128                   # partitions
NKC = D_IN // P           # 32 contraction chunks for the down proj
NPAIR = NKC // 2          # 16 even/odd chunk pairs
QW = 512                  # matmul free width (one PSUM bank of f32)
NQ = NT // QW             # 4 free-dim tiles
NOC = D_OUT // P          # 32 output-row chunks for the up proj

_CACHE = {}


def _build_program():
    import concourse.bacc as bacc
    import concourse.mybir as mybir
    from concourse import tile

    f32 = mybir.dt.float32
    bf16 = mybir.dt.bfloat16
    nc = bacc.Bacc()

    ht = nc.declare_dram_parameter("ht", [D_IN, NT], bf16, isOutput=False)
    dwt = nc.declare_dram_parameter("dwt", [P, NKC * RANK], bf16, isOutput=False)
    upw2 = nc.declare_dram_parameter("upw2", [P, D_OUT], bf16, isOutput=False)
    maskt = nc.declare_dram_parameter("maskt", [P, NT], bf16, isOutput=False)
    outt = nc.declare_dram_parameter("outt", [D_OUT, NT], bf16, isOutput=True)

    with tile.TileContext(nc) as tc:
        with (
            tc.tile_pool(name="const", bufs=1) as const,
            tc.tile_pool(name="hch", bufs=5) as hch_pool,
            tc.tile_pool(name="res", bufs=1) as res_pool,
            tc.tile_pool(name="outsb", bufs=3) as out_pool,
            tc.tile_pool(name="psum_dn", bufs=1, space="PSUM") as psum_dn_pool,
            tc.tile_pool(name="psum_up", bufs=3, space="PSUM") as psum_up_pool,
        ):
            dwt_sb = const.tile([P, NKC * RANK], bf16, name="dwt_sb")
            upw2_sb = const.tile([P, D_OUT], bf16, name="upw2_sb")
            maskt_sb = const.tile([P, NT], bf16, name="maskt_sb")
            nc.sync.dma_start(out=dwt_sb[:], in_=dwt[:, :])

            # down accumulators: [128, 512] f32 = one PSUM bank each;
            # partitions 0-63 accumulate even ki chunks, 64-127 odd chunks
            dn = [
                psum_dn_pool.tile([P, QW], f32, name=f"dn_{q}")
                for q in range(NQ)
            ]

            for pr in range(NPAIR):
                hcs = []
                for j in range(2):
                    ki = 2 * pr + j
                    hc = hch_pool.tile([P, NT], bf16, name="hc")
                    nc.sync.dma_start(out=hc[:], in_=ht[ki * P:(ki + 1) * P, :])
                    hcs.append(hc)
                if pr == 1:
                    # needed only from the mask/up phase (~50 us in):
                    # slot the loads behind the first two chunk pairs
                    nc.sync.dma_start(out=upw2_sb[:], in_=upw2[:, :])
                    nc.sync.dma_start(out=maskt_sb[:], in_=maskt[:, :])
                for q in range(NQ):
                    for j in range(2):
                        ki = 2 * pr + j
                        # even -> psum partitions 0-63 (array cols 0-63),
                        # odd -> 64-127; the two column-tile streams run
                        # concurrently on the PE
                        nc.tensor.matmul(
                            dn[q][j * RANK:(j + 1) * RANK, :],
                            lhsT=dwt_sb[:, ki * RANK:(ki + 1) * RANK],
                            rhs=hcs[j][:, q * QW:(q + 1) * QW],
                            start=(pr == 0),
                            stop=(pr == NPAIR - 1),
                            skip_group_check=True,
                        )

            # evict downT psum -> sbuf bf16 fused with the top-k mask
            resT = res_pool.tile([P, NT], bf16, name="resT")
            for q in range(NQ):
                nc.vector.tensor_mul(
                    resT[:, q * QW:(q + 1) * QW],
                    maskt_sb[:, q * QW:(q + 1) * QW],
                    dn[q][:],
                )

            # up-proj: outT[oc] = upw2.T @ resT with K=128 (the stacked
            # even/odd partials sum inside the contraction)
            for oc in range(NOC):
                osb = out_pool.tile([P, NT], bf16, name="osb")
                for q in range(NQ):
                    pu = psum_up_pool.tile([P, QW], f32, name="pu")
                    nc.tensor.matmul(
                        pu[:],
                        lhsT=upw2_sb[:, oc * P:(oc + 1) * P],
                        rhs=resT[:, q * QW:(q + 1) * QW],
                        start=True,
                        stop=True,
                    )
                    if q % 2 == 0:
                        nc.scalar.copy(out=osb[:, q * QW:(q + 1) * QW], in_=pu[:])
                    else:
                        nc.vector.tensor_copy(
                            out=osb[:, q * QW:(q + 1) * QW], in_=pu[:]
                        )
                nc.sync.dma_start(out=outt[oc * P:(oc + 1) * P, :], in_=osb[:])

    nc.finalize()
    return nc


def _get_program():
    if "nc" not in _CACHE:
        _CACHE["nc"] = _build_program()
    return _CACHE["nc"]


def prepare_in_maps(hidden_states, down_w, up_w, top_k_values, top_k_indices):
    h = np.ascontiguousarray(hidden_states, dtype=np.float32).astype(BF16)
    dw = np.ascontiguousarray(down_w, dtype=np.float32).astype(BF16)
    uw = np.ascontiguousarray(up_w, dtype=np.float32).astype(BF16)
    vals = np.ascontiguousarray(top_k_values, dtype=np.float32)
    idx = np.asarray(top_k_indices).astype(np.int64)

    # dwt[i, ki*64 + r] = dw[r, ki*128 + i]
    dwt = np.ascontiguousarray(
        dw.reshape(RANK, NKC, P).transpose(2, 1, 0).reshape(P, NKC * RANK)
    )
    # up weights transposed and stacked twice: K=128 contraction sums the
    # even-ki (partitions 0-63) and odd-ki (64-127) down partials
    upw2 = np.ascontiguousarray(np.vstack([uw.T, uw.T]))  # [128, 4096]

    rows = np.arange(NT)[:, None]
    in_maps = []
    for c in range(NCORES):
        s = slice(c * NT, (c + 1) * NT)
        ht = np.ascontiguousarray(h[s].T)  # [4096, 2048]
        m = np.zeros((NT, RANK), dtype=np.float32)
        m[rows, idx[s]] = vals[s]
        mt = m.T.astype(BF16)  # [64, 2048]
        in_maps.append(
            {
                "ht": ht,
                "dwt": dwt,
                "upw2": upw2,
                "maskt": np.ascontiguousarray(np.vstack([mt, mt])),  # [128, 2048]
            }
        )
    return in_maps


def gather_output(results):
    # each core returns outT [4096, 2048] bf16; un-transpose + upcast
    return np.concatenate(
        [np.asarray(r["outt"]).T.astype(np.float32) for r in results], axis=0
    )


def kernel(hidden_states, down_w, up_w, top_k_values, top_k_indices, **_kw):
    from concourse.bass_utils import run_bass_kernel_spmd

    nc = _get_program()
    in_maps = prepare_in_maps(
        hidden_states, down_w, up_w, top_k_values, top_k_indices
    )
    res = run_bass_kernel_spmd(nc, in_maps, core_ids=list(range(NCORES)))
    return gather_output(res.results)


# revision 5
# speedup vs baseline: 2.0876x; 1.0503x over previous
"""MoE LoRA linear layer kernel for Trainium2, data-parallel over 8 NeuronCores.

Math (per token n):
    down = h @ down_w.T                      [N, 64]
    mask[n, r] = val[n, k] if idx[n, k] == r else 0   (indices distinct per row)
    out = (down * mask) @ up_w.T             [N, 4096]

Sharding: tokens split 8 ways (2048/core); LoRA weights replicated.

Strategy (v4): device does two matmul passes + one fused DVE multiply;
all layout work happens in the host packer, all traffic is bf16
(accumulation in f32 PSUM). DMA roofline ~34 MiB/core @ ~320 GB/s.

  * h is pre-transposed AND ki-pair-packed on the host
    (ht2[pr*128+p, j*2048+n] = h[n, (2pr+j)*128+p]) so each of the 16
    loads is 1 MiB of contiguous 8 KiB descriptors.
  * down-proj: even ki chunks write PSUM partitions 0-63, odd ki
    chunks partitions 64-127 (128x64 column-tiled array mode, two
    concurrent tile streams) into ONE [128, 2048] 4-bank accumulator.
  * up-proj contracts K=128 against host-duplicated up weights
    (upw2 = [upT; upT]): the even/odd partial sums combine inside the
    matmul -- full 128x128 array, FWL weight loads.
  * the top-k scatter mask is a dense host-built maskT (bf16,
    replicated to 128 partitions); masking fuses with the PSUM->SBUF
    eviction on the DVE.
  * up-proj emits outT (stationary weights, transposed output),
    oc-pair-packed to make 1 MiB stores; the host unpacks + transposes
    while gathering the 8 shards.
"""

import sys

for p in ("/opt/trn_rl_repo", "/opt/pypackages"):
    if p not in sys.path:
        sys.path.insert(0, p)

import ml_dtypes
import numpy as np

BF16 = ml_dtypes.bfloat16

N, D_IN, D_OUT, RANK, TOPK = 16384, 4096, 4096, 64, 8
NCORES = 8
NT = N // NCORES          # tokens per core = 2048
P = 128                   # partitions
NKC = D_IN // P           # 32 contraction chunks for the down proj
NPAIR = NKC // 2          # 16 even/odd chunk pairs
QW = 512                  # matmul free width (one PSUM bank of f32)
NQ = NT // QW             # 4 free-dim tiles
NOCP = D_OUT // (2 * P)   # 16 output-row-chunk pairs for the up proj

_CACHE = {}


def _build_program():
    import concourse.bacc as bacc
    import concourse.mybir as mybir
    from concourse import tile

    f32 = mybir.dt.float32
    bf16 = mybir.dt.bfloat16
    nc = bacc.Bacc()

    ht2 = nc.declare_dram_parameter("ht2", [D_IN // 2, 2 * NT], bf16, isOutput=False)
    dwt = nc.declare_dram_parameter("dwt", [P, NKC * RANK], bf16, isOutput=False)
    upw2 = nc.declare_dram_parameter("upw2", [P, D_OUT], bf16, isOutput=False)
    maskt = nc.declare_dram_parameter("maskt", [P, NT], bf16, isOutput=False)
    outt2 = nc.declare_dram_parameter("outt2", [D_OUT // 2, 2 * NT], bf16, isOutput=True)

    with tile.TileContext(nc) as tc:
        with (
            tc.tile_pool(name="const", bufs=1) as const,
            tc.tile_pool(name="hch", bufs=3) as hch_pool,
            tc.tile_pool(name="res", bufs=1) as res_pool,
            tc.tile_pool(name="outsb", bufs=3) as out_pool,
        ):
            dwt_sb = const.tile([P, NKC * RANK], bf16, name="dwt_sb")
            upw2_sb = const.tile([P, D_OUT], bf16, name="upw2_sb")
            maskt_sb = const.tile([P, NT], bf16, name="maskt_sb")
            nc.sync.dma_start(out=dwt_sb[:], in_=dwt[:, :])

            resT = res_pool.tile([P, NT], bf16, name="resT")

            with tc.tile_pool(name="psum_dn", bufs=1, space="PSUM") as psum_dn_pool:
                # single 4-bank accumulator: partitions 0-63 = even ki
                # chunks, 64-127 = odd ki chunks
                dn = psum_dn_pool.tile([P, NT], f32, name="dn")

                for pr in range(NPAIR):
                    hc = hch_pool.tile([P, 2 * NT], bf16, name="hc")
                    nc.sync.dma_start(
                        out=hc[:], in_=ht2[pr * P:(pr + 1) * P, :]
                    )
                    if pr == 1:
                        # needed only from the mask/up phase (~55 us in):
                        # slot the loads behind the first two chunk pairs
                        nc.sync.dma_start(out=upw2_sb[:], in_=upw2[:, :])
                        nc.sync.dma_start(out=maskt_sb[:], in_=maskt[:, :])
                    for q in range(NQ):
                        for j in range(2):
                            ki = 2 * pr + j
                            # even -> psum partitions 0-63 (array cols
                            # 0-63), odd -> 64-127; the two column-tile
                            # streams run concurrently on the PE
                            nc.tensor.matmul(
                                dn[j * RANK:(j + 1) * RANK,
                                   q * QW:(q + 1) * QW],
                                lhsT=dwt_sb[:, ki * RANK:(ki + 1) * RANK],
                                rhs=hc[:, j * NT + q * QW:j * NT + (q + 1) * QW],
                                start=(pr == 0),
                                stop=(pr == NPAIR - 1),
                                skip_group_check=True,
                            )

                # evict downT psum -> sbuf bf16 fused with the top-k mask
                for half in range(2):
                    cols = slice(half * NT // 2, (half + 1) * NT // 2)
                    nc.vector.tensor_mul(
                        resT[:, cols], maskt_sb[:, cols], dn[:, cols]
                    )

            # up-proj: outT[oc] = upw2.T @ resT with K=128 (the stacked
            # even/odd partials sum inside the contraction)
            with tc.tile_pool(name="psum_up", bufs=3, space="PSUM") as psum_up_pool:
                for ocp in range(NOCP):
                    osb = out_pool.tile([P, 2 * NT], bf16, name="osb")
                    for oc_in in range(2):
                        oc = 2 * ocp + oc_in
                        for qq in range(2):
                            pu = psum_up_pool.tile([P, 2 * QW], f32, name="pu")
                            for k in range(2):
                                q = 2 * qq + k
                                nc.tensor.matmul(
                                    pu[:, k * QW:(k + 1) * QW],
                                    lhsT=upw2_sb[:, oc * P:(oc + 1) * P],
                                    rhs=resT[:, q * QW:(q + 1) * QW],
                                    start=True,
                                    stop=True,
                                )
                            dst = osb[:, oc_in * NT + qq * 2 * QW:
                                      oc_in * NT + (qq + 1) * 2 * QW]
                            if (oc_in + qq) % 2 == 0:
                                nc.scalar.copy(out=dst, in_=pu[:])
                            else:
                                nc.vector.tensor_copy(out=dst, in_=pu[:])
                    nc.sync.dma_start(
                        out=outt2[ocp * P:(ocp + 1) * P, :], in_=osb[:]
                    )

    nc.finalize()
    return nc


def _get_program():
    if "nc" not in _CACHE:
        _CACHE["nc"] = _build_program()
    return _CACHE["nc"]


def prepare_in_maps(hidden_states, down_w, up_w, top_k_values, top_k_indices):
    h = np.ascontiguousarray(hidden_states, dtype=np.float32).astype(BF16)
    dw = np.ascontiguousarray(down_w, dtype=np.float32).astype(BF16)
    uw = np.ascontiguousarray(up_w, dtype=np.float32).astype(BF16)
    vals = np.ascontiguousarray(top_k_values, dtype=np.float32)
    idx = np.asarray(top_k_indices).astype(np.int64)

    # dwt[i, ki*64 + r] = dw[r, ki*128 + i]
    dwt = np.ascontiguousarray(
        dw.reshape(RANK, NKC, P).transpose(2, 1, 0).reshape(P, NKC * RANK)
    )
    # up weights transposed and stacked twice: K=128 contraction sums the
    # even-ki (partitions 0-63) and odd-ki (64-127) down partials
    upw2 = np.ascontiguousarray(np.vstack([uw.T, uw.T]))  # [128, 4096]

    rows = np.arange(NT)[:, None]
    in_maps = []
    for c in range(NCORES):
        s = slice(c * NT, (c + 1) * NT)
        # ht2[pr*128+p, j*2048+n] = h[s][n, (2pr+j)*128+p]
        ht = h[s].T  # [4096, 2048]
        ht2 = np.ascontiguousarray(
            ht.reshape(NPAIR, 2, P, NT).transpose(0, 2, 1, 3).reshape(D_IN // 2, 2 * NT)
        )
        m = np.zeros((NT, RANK), dtype=np.float32)
        m[rows, idx[s]] = vals[s]
        mt = m.T.astype(BF16)  # [64, 2048]
        in_maps.append(
            {
                "ht2": ht2,
                "dwt": dwt,
                "upw2": upw2,
                "maskt": np.ascontiguousarray(np.vstack([mt, mt])),  # [128, 2048]
            }
        )
    return in_maps


def gather_output(results):
    # each core returns outt2 [2048, 4096] bf16 with
    # outt2[ocp*128+p, oc_in*2048+n] = outT[(2*ocp+oc_in)*128+p, n];
    # unpack to outT [4096, 2048], transpose, upcast
    outs = []
    for r in results:
        o2 = np.asarray(r["outt2"])
        outT = (
            o2.reshape(NOCP, P, 2, NT)
            .transpose(0, 2, 1, 3)
            .reshape(D_OUT, NT)
        )
        outs.append(outT.T.astype(np.float32))
    return np.concatenate(outs, axis=0)


def kernel(hidden_states, down_w, up_w, top_k_values, top_k_indices, **_kw):
    from concourse.bass_utils import run_bass_kernel_spmd

    nc = _get_program()
    in_maps = prepare_in_maps(
        hidden_states, down_w, up_w, top_k_values, top_k_indices
    )
    res = run_bass_kernel_spmd(nc, in_maps, core_ids=list(range(NCORES)))
    return gather_output(res.results)


# revision 10
# speedup vs baseline: 2.1314x; 1.0210x over previous
"""MoE LoRA linear layer kernel for Trainium2, data-parallel over 8 NeuronCores.

Math (per token n):
    down = h @ down_w.T                      [N, 64]
    mask[n, r] = val[n, k] if idx[n, k] == r else 0   (indices distinct per row)
    out = (down * mask) @ up_w.T             [N, 4096]

Sharding: tokens split 8 ways (2048/core); LoRA weights replicated.

Strategy (v4): device does two matmul passes + one fused DVE multiply;
all layout work happens in the host packer, all traffic is bf16
(accumulation in f32 PSUM). DMA roofline ~34 MiB/core @ ~320 GB/s.

  * h is pre-transposed AND ki-pair-packed on the host
    (ht2[pr*128+p, j*2048+n] = h[n, (2pr+j)*128+p]) so each of the 16
    loads is 1 MiB of contiguous 8 KiB descriptors.
  * down-proj: even ki chunks write PSUM partitions 0-63, odd ki
    chunks partitions 64-127 (128x64 column-tiled array mode, two
    concurrent tile streams) into ONE [128, 2048] 4-bank accumulator.
  * up-proj contracts K=128 against host-duplicated up weights
    (upw2 = [upT; upT]): the even/odd partial sums combine inside the
    matmul -- full 128x128 array, FWL weight loads.
  * the top-k scatter mask is a dense host-built maskT (bf16,
    replicated to 128 partitions); masking fuses with the PSUM->SBUF
    eviction on the DVE.
  * up-proj emits outT (stationary weights, transposed output),
    oc-pair-packed to make 1 MiB stores; the host unpacks + transposes
    while gathering the 8 shards.
"""

import sys

for p in ("/opt/trn_rl_repo", "/opt/pypackages"):
    if p not in sys.path:
        sys.path.insert(0, p)

import ml_dtypes
import numpy as np

BF16 = ml_dtypes.bfloat16

N, D_IN, D_OUT, RANK, TOPK = 16384, 4096, 4096, 64, 8
NCORES = 8
NT = N // NCORES          # tokens per core = 2048
P = 128                   # partitions
NKC = D_IN // P           # 32 contraction chunks for the down proj
NPAIR = NKC // 2          # 16 even/odd chunk pairs
QW = 512                  # matmul free width (one PSUM bank of f32)
NQ = NT // QW             # 4 free-dim tiles
NOCP = D_OUT // (2 * P)   # 16 output-row-chunk pairs for the up proj

_CACHE = {}


def _build_program():
    import concourse.bacc as bacc
    import concourse.mybir as mybir
    from concourse import tile

    f32 = mybir.dt.float32
    bf16 = mybir.dt.bfloat16
    nc = bacc.Bacc()

    ht4 = nc.declare_dram_parameter("ht4", [D_IN // 4, 4 * NT], bf16, isOutput=False)
    dwt = nc.declare_dram_parameter("dwt", [P, NKC * RANK], bf16, isOutput=False)
    upw2 = nc.declare_dram_parameter("upw2", [P, D_OUT], bf16, isOutput=False)
    maskt = nc.declare_dram_parameter("maskt", [P, NT], bf16, isOutput=False)
    outt2 = nc.declare_dram_parameter("outt2", [D_OUT // 2, 2 * NT], bf16, isOutput=True)

    with tile.TileContext(nc) as tc:
        with (
            tc.tile_pool(name="const", bufs=1) as const,
            tc.tile_pool(name="hch", bufs=3) as hch_pool,
            tc.tile_pool(name="res", bufs=1) as res_pool,
            tc.tile_pool(name="outsb", bufs=3) as out_pool,
        ):
            dwt_sb = const.tile([P, NKC * RANK], bf16, name="dwt_sb")
            upw2_sb = const.tile([P, D_OUT], bf16, name="upw2_sb")
            maskt_sb = const.tile([P, NT], bf16, name="maskt_sb")
            nc.sync.dma_start(out=dwt_sb[:], in_=dwt[:, :])

            resT = res_pool.tile([P, NT], bf16, name="resT")

            with tc.tile_pool(name="psum_dn", bufs=1, space="PSUM") as psum_dn_pool:
                # single 4-bank accumulator: partitions 0-63 = even ki
                # chunks, 64-127 = odd ki chunks
                dn = psum_dn_pool.tile([P, NT], f32, name="dn")

                NQUAD = NKC // 4
                for qr in range(NQUAD):
                    hc = hch_pool.tile([P, 4 * NT], bf16, name="hc")
                    nc.sync.dma_start(
                        out=hc[:], in_=ht4[qr * P:(qr + 1) * P, :]
                    )
                    if qr == 1:
                        # needed only from the mask/up phase (~55 us in):
                        # slot the loads behind the first chunk quads
                        nc.sync.dma_start(out=upw2_sb[:], in_=upw2[:, :])
                        nc.sync.dma_start(out=maskt_sb[:], in_=maskt[:, :])
                    for q in range(NQ):
                        for j4 in range(4):
                            ki = 4 * qr + j4
                            j = ki % 2
                            # even ki -> psum partitions 0-63 (array cols
                            # 0-63), odd -> 64-127; the two column-tile
                            # streams run concurrently on the PE
                            nc.tensor.matmul(
                                dn[j * RANK:(j + 1) * RANK,
                                   q * QW:(q + 1) * QW],
                                lhsT=dwt_sb[:, ki * RANK:(ki + 1) * RANK],
                                rhs=hc[:, j4 * NT + q * QW:j4 * NT + (q + 1) * QW],
                                start=(qr == 0 and j4 < 2),
                                stop=(qr == NQUAD - 1 and j4 >= 2),
                                skip_group_check=True,
                            )

                # evict downT psum -> sbuf bf16 fused with the top-k mask;
                # quarter granularity so the up phase starts after q0
                for q in range(NQ):
                    cols = slice(q * QW, (q + 1) * QW)
                    nc.vector.tensor_mul(
                        resT[:, cols], maskt_sb[:, cols], dn[:, cols]
                    )

            # up-proj: outT[oc] = upw2.T @ resT with K=128 (the stacked
            # even/odd partials sum inside the contraction)
            with tc.tile_pool(name="psum_up", bufs=4, space="PSUM") as psum_up_pool:
                for ocp in range(NOCP):
                    osb = out_pool.tile([P, 2 * NT], bf16, name="osb")
                    for oc_in in range(2):
                        oc = 2 * ocp + oc_in
                        for qq in range(2):
                            pu = psum_up_pool.tile([P, 2 * QW], f32, name="pu")
                            for k in range(2):
                                q = 2 * qq + k
                                nc.tensor.matmul(
                                    pu[:, k * QW:(k + 1) * QW],
                                    lhsT=upw2_sb[:, oc * P:(oc + 1) * P],
                                    rhs=resT[:, q * QW:(q + 1) * QW],
                                    start=True,
                                    stop=True,
                                )
                            dst = osb[:, oc_in * NT + qq * 2 * QW:
                                      oc_in * NT + (qq + 1) * 2 * QW]
                            if (oc_in + qq) % 2 == 0:
                                nc.scalar.copy(out=dst, in_=pu[:])
                            else:
                                nc.vector.tensor_copy(out=dst, in_=pu[:])
                    nc.sync.dma_start(
                        out=outt2[ocp * P:(ocp + 1) * P, :], in_=osb[:]
                    )

    nc.finalize()
    return nc


def _get_program():
    if "nc" not in _CACHE:
        _CACHE["nc"] = _build_program()
    return _CACHE["nc"]


def prepare_in_maps(hidden_states, down_w, up_w, top_k_values, top_k_indices):
    h = np.ascontiguousarray(hidden_states, dtype=np.float32).astype(BF16)
    dw = np.ascontiguousarray(down_w, dtype=np.float32).astype(BF16)
    uw = np.ascontiguousarray(up_w, dtype=np.float32).astype(BF16)
    vals = np.ascontiguousarray(top_k_values, dtype=np.float32)
    idx = np.asarray(top_k_indices).astype(np.int64)

    # dwt[i, ki*64 + r] = dw[r, ki*128 + i]
    dwt = np.ascontiguousarray(
        dw.reshape(RANK, NKC, P).transpose(2, 1, 0).reshape(P, NKC * RANK)
    )
    # up weights transposed and stacked twice: K=128 contraction sums the
    # even-ki (partitions 0-63) and odd-ki (64-127) down partials
    upw2 = np.ascontiguousarray(np.vstack([uw.T, uw.T]))  # [128, 4096]

    rows = np.arange(NT)[:, None]
    in_maps = []
    for c in range(NCORES):
        s = slice(c * NT, (c + 1) * NT)
        # ht4[qr*128+p, j4*2048+n] = h[s][n, (4qr+j4)*128+p]
        ht = h[s].T  # [4096, 2048]
        ht4 = np.ascontiguousarray(
            ht.reshape(NKC // 4, 4, P, NT).transpose(0, 2, 1, 3).reshape(D_IN // 4, 4 * NT)
        )
        m = np.zeros((NT, RANK), dtype=np.float32)
        m[rows, idx[s]] = vals[s]
        mt = m.T.astype(BF16)  # [64, 2048]
        in_maps.append(
            {
                "ht4": ht4,
                "dwt": dwt,
                "upw2": upw2,
                "maskt": np.ascontiguousarray(np.vstack([mt, mt])),  # [128, 2048]
            }
        )
    return in_maps


def gather_output(results):
    # each core returns outt2 [2048, 4096] bf16 with
    # outt2[ocp*128+p, oc_in*2048+n] = outT[(2*ocp+oc_in)*128+p, n];
    # unpack to outT [4096, 2048], transpose, upcast
    outs = []
    for r in results:
        o2 = np.asarray(r["outt2"])
        outT = (
            o2.reshape(NOCP, P, 2, NT)
            .transpose(0, 2, 1, 3)
            .reshape(D_OUT, NT)
        )
        outs.append(outT.T.astype(np.float32))
    return np.concatenate(outs, axis=0)


def kernel(hidden_states, down_w, up_w, top_k_values, top_k_indices, **_kw):
    from concourse.bass_utils import run_bass_kernel_spmd

    nc = _get_program()
    in_maps = prepare_in_maps(
        hidden_states, down_w, up_w, top_k_values, top_k_indices
    )
    res = run_bass_kernel_spmd(nc, in_maps, core_ids=list(range(NCORES)))
    return gather_output(res.results)
